# revision 2
# baseline (speedup 1.0000x reference)
"""Trainium2 Bass kernel: causal multi-head self-attention with RoPE.

Problem: x[2,2048,1024], 16 heads, d_k=64, causal, RoPE(theta=1e4),
out = (softmax(rope(Q)rope(K)^T/8) V) WO^T.

Sharding (8 cores): data-parallel over batch (2) x head-parallel over
head groups (4 heads per core).  Each core computes Q/K/V projections
for its 4 heads, flash-style causal attention, and a partial output
projection over its 256 channels; the host sums the 4 partials per
batch element.

Device layouts (per core, all bf16 except PSUM):
  xt  [1024,2048]  x[b]^T (d_model on partitions), shipped pre-chunked
      in the exact SBUF layout so every DMA is contiguous.
  Qt/Kt tiles [128,2048]: 2 heads each, per head rows = [32 even-dim,
      32 odd-dim] (host permutes W_Q/W_K columns) so RoPE is pure
      row-block ops; scores are permutation-invariant.
  V   [128,16,4,65]: natural [s,d] layout per 128-row s-block, 65th
      column of ones => P@[V|1] yields softmax denominators for free.
  scores computed transposed ([keys,queries]) so P^T feeds P@V with the
      contraction (keys) on partitions.  Causal masking: an identity
      matmul preloads -1e5 into the key>query region of the scores psum
      (keeps the mask off the Vector engine / out of the exp->PV chain);
      PV skips fully-masked leading columns of diagonal key blocks.
  softmax normalization is deferred: unnormalized head outputs plus the
      denominator rows are staged, then one fast-reciprocal + an
      indicator-matmul broadcast rescales everything at the tail,
      pipelined per query-slice with the output projection.
"""

import os
import sys

for _p in ("/opt/trn_rl_repo",):
    if _p not in sys.path:
        sys.path.insert(0, _p)

import numpy as np
import ml_dtypes

BF16 = ml_dtypes.bfloat16

D = 1024
S = 2048
H = 16
DK = 64
HPC = 4          # heads per core
NCORES = 8
THETA = 10000.0

_COMPILED = {}


def _build_nc():
    import concourse.bass as bass  # noqa: F401
    import concourse.bacc as bacc
    import concourse.mybir as mybir
    import concourse.tile as tile

    bf16 = mybir.dt.bfloat16
    f32 = mybir.dt.float32
    Exp = mybir.ActivationFunctionType.Exp

    nc = bacc.Bacc(
        "TRN2", target_bir_lowering=False, debug=False, num_devices=NCORES
    )
    xt_d = nc.declare_dram_parameter("xt", [4, 128, 8, 512], bf16, isOutput=False)
    wq_d = nc.declare_dram_parameter("wq", [128, 8, 256], bf16, isOutput=False)
    wk_d = nc.declare_dram_parameter("wk", [128, 8, 256], bf16, isOutput=False)
    wv_d = nc.declare_dram_parameter("wv", [128, 8, 256], bf16, isOutput=False)
    wo_d = nc.declare_dram_parameter("wo", [128, 2, D], bf16, isOutput=False)
    cos_d = nc.declare_dram_parameter("cosb", [128, S], bf16, isOutput=False)
    sin_d = nc.declare_dram_parameter("sinb", [128, S], bf16, isOutput=False)
    msk_d = nc.declare_dram_parameter("msk", [128, 4, 512], bf16, isOutput=False)
    eye_d = nc.declare_dram_parameter("eye", [128, 128], bf16, isOutput=False)
    ind_d = nc.declare_dram_parameter("ind", [40, 4, 128], bf16, isOutput=False)
    out_d = nc.declare_dram_parameter("out", [S, D], bf16, isOutput=True)

    with tile.TileContext(nc) as tc:
        with tc.tile_pool(name="const", bufs=1) as const:
            x_sb = const.tile([128, 8, S], bf16)
            wq_sb = const.tile([128, 8, 256], bf16)
            wk_sb = const.tile([128, 8, 256], bf16)
            wv_sb = const.tile([128, 8, 256], bf16)
            wo_sb = const.tile([128, 2, D], bf16)
            cos_sb = const.tile([128, S], bf16)
            sin_sb = const.tile([128, S], bf16)
            msk_sb = const.tile([128, 4, 512], bf16)
            eye_sb = const.tile([128, 128], bf16)
            ind_sb = const.tile([40, 4, 128], bf16)
            v_sb = const.tile([128, 16, 4, 65], bf16)
            qraw = [const.tile([128, S], bf16, name=f"qraw{i}") for i in range(2)]
            kraw = [const.tile([128, S], bf16, name=f"kraw{i}") for i in range(2)]
            qrot = [const.tile([128, S], bf16, name=f"qrot{i}") for i in range(2)]
            krot = [const.tile([128, S], bf16, name=f"krot{i}") for i in range(2)]
            at = [const.tile([128, S], bf16, name=f"at{i}") for i in range(2)]

            # x slices stream on the HW-DGE (sync) queue; everything else is
            # issued in parallel from the gpsimd queue
            for nsl in range(4):
                nc.sync.dma_start(
                    x_sb[:, :, nsl * 512:(nsl + 1) * 512], xt_d[nsl]
                )
            nc.gpsimd.dma_start(wq_sb[:], wq_d[:])
            nc.gpsimd.dma_start(wk_sb[:], wk_d[:])
            nc.gpsimd.dma_start(wv_sb[:], wv_d[:])
            nc.gpsimd.dma_start(cos_sb[:], cos_d[:])
            nc.gpsimd.dma_start(sin_sb[:], sin_d[:])
            nc.gpsimd.dma_start(msk_sb[:], msk_d[:])
            nc.gpsimd.dma_start(eye_sb[:], eye_d[:])
            nc.gpsimd.dma_start(ind_sb[:], ind_d[:])
            nc.gpsimd.dma_start(wo_sb[:], wo_d[:])
            nc.vector.memset(v_sb[:, :, :, 64:65], 1.0)

            # ---- phase 1: Q/K/V projections + RoPE ----
            with tc.tile_pool(name="pj", bufs=4, space="PSUM") as pjp, \
                 tc.tile_pool(name="pvps", bufs=2, space="PSUM") as pvps:
                for nsl in range(4):
                    for w_sb, raw in ((wq_sb, qraw), (wk_sb, kraw)):
                        for ot in range(2):
                            ps = pjp.tile([128, 512], f32, tag="pj", name="pj")
                            for c in range(8):
                                nc.tensor.matmul(
                                    ps[:],
                                    w_sb[:, c, ot * 128:(ot + 1) * 128],
                                    x_sb[:, c, nsl * 512:(nsl + 1) * 512],
                                    start=(c == 0), stop=(c == 7),
                                )
                            nc.vector.tensor_copy(
                                raw[ot][:, nsl * 512:(nsl + 1) * 512], ps[:]
                            )
                for sb in range(16):
                    ps = pvps.tile([128, 256], f32, tag="pv", name="pv")
                    for c in range(8):
                        nc.tensor.matmul(
                            ps[:],
                            x_sb[:, c, sb * 128:(sb + 1) * 128],
                            wv_sb[:, c, :],
                            start=(c == 0), stop=(c == 7),
                        )
                    nc.vector.tensor_copy(
                        v_sb[:, sb, :, 0:64],
                        ps[:].rearrange("p (h d) -> p h d", h=4),
                    )
                with tc.tile_pool(name="rope", bufs=2) as rp:
                    for raw, rot in ((qraw, qrot), (kraw, krot)):
                        for ot in range(2):
                            sw = rp.tile([128, S], bf16, tag="sw", name="sw")
                            t1 = rp.tile([128, S], bf16, tag="t1", name="t1")
                            for blk in range(4):
                                src = blk ^ 1
                                nc.sync.dma_start(
                                    sw[blk * 32:(blk + 1) * 32, :],
                                    raw[ot][src * 32:(src + 1) * 32, :],
                                )
                            nc.vector.tensor_mul(t1[:], raw[ot][:], cos_sb[:])
                            nc.vector.tensor_mul(sw[:], sw[:], sin_sb[:])
                            nc.vector.tensor_add(rot[ot][:], t1[:], sw[:])

            # ---- phase 2: causal attention (scores transposed) ----
            den_sb = const.tile([40, 512], bf16)
            rc = const.tile([40, 512], f32)
            rcb = const.tile([40, 512], bf16)
            atn = [const.tile([128, 4, 512], bf16, name=f"atn{i}")
                   for i in range(2)]
            with tc.tile_pool(name="ps_s", bufs=3, space="PSUM") as psc, \
                 tc.tile_pool(name="ps_o", bufs=2, space="PSUM") as pso, \
                 tc.tile_pool(name="pp", bufs=3) as ppool, \
                 tc.tile_pool(name="nrm", bufs=3) as nrm:
                for h in range(HPC):
                    ot, hl = divmod(h, 2)
                    qr, kr = qrot[ot], krot[ot]
                    r0 = hl * 64
                    for j in range(4):
                        nkb = 4 * (j + 1)
                        po = pso.tile([65, 512], f32, tag="po", name="po")
                        for g0 in range(0, nkb, 2):
                            G = min(2, nkb - g0)
                            sp = psc.tile([128, 1024], f32, tag="sc", name="sp")
                            pt = ppool.tile([128, 1024], bf16, tag="pt", name="pt")
                            for i in range(G):
                                kb = g0 + i
                                dg = kb - 4 * j
                                if dg >= 0:
                                    # causal mask: preload psum with -1e5 in
                                    # the key>query region via identity matmul
                                    nc.tensor.matmul(
                                        sp[:, i * 512:(i + 1) * 512],
                                        eye_sb[:],
                                        msk_sb[:, dg, :],
                                        start=True, stop=False,
                                    )
                                nc.tensor.matmul(
                                    sp[:, i * 512:(i + 1) * 512],
                                    kr[r0:r0 + 64, kb * 128:(kb + 1) * 128],
                                    qr[r0:r0 + 64, j * 512:(j + 1) * 512],
                                    start=(dg < 0), stop=True,
                                )
                            nc.scalar.activation(
                                pt[:, 0:G * 512], sp[:, 0:G * 512], Exp, scale=0.125
                            )
                            for i in range(G):
                                kb = g0 + i
                                dg = kb - 4 * j
                                # cols < dg*128 of a diagonal block are fully
                                # masked (exactly 0 after exp): PV skips them
                                c0 = dg * 128 if dg > 0 else 0
                                nc.tensor.matmul(
                                    po[:, c0:512],
                                    v_sb[:, kb, h, 0:65],
                                    pt[:, i * 512 + c0:(i + 1) * 512],
                                    start=(kb == 0), stop=(kb == nkb - 1),
                                )
                        # stage unnormalized out + denominator, release po fast
                        tm = nrm.tile([65, 512], bf16, tag="tm", name="tm")
                        nc.vector.tensor_copy(tm[:], po[:])
                        nc.sync.dma_start(
                            at[ot][r0:r0 + 64, j * 512:(j + 1) * 512], tm[0:64, :]
                        )
                        dr = ot * 32 + hl * 4 + j
                        nc.sync.dma_start(den_sb[dr:dr + 1, :], tm[64:65, :])

            # ---- tail: normalization + output projection, pipelined per jsl ----
            with tc.tile_pool(name="ps_r", bufs=2, space="PSUM") as psr, \
                 tc.tile_pool(name="ps_f", bufs=4, space="PSUM") as psf, \
                 tc.tile_pool(name="ost", bufs=4) as ost:
                denf = ost.tile([40, 512], f32, tag="denf", name="denf")
                nc.vector.tensor_copy(denf[:], den_sb[:])
                nc.vector.reciprocal_approx_fast(rc[:], denf[:])
                nc.vector.tensor_copy(rcb[:], rc[:])
                for jsl in range(4):
                    for ot in range(2):
                        rbp = psr.tile([128, 512], f32, tag="rb", name="rb")
                        nc.tensor.matmul(
                            rbp[:], ind_sb[ot * 32:ot * 32 + 8, jsl, :],
                            rcb[ot * 32:ot * 32 + 8, :], start=True, stop=True,
                        )
                        nc.vector.tensor_mul(
                            atn[ot][:, jsl, :],
                            at[ot][:, jsl * 512:(jsl + 1) * 512],
                            rbp[:],
                        )
                    for sbi in range(4):
                        sb = jsl * 4 + sbi
                        for osl in range(2):
                            pf = psf.tile([128, 512], f32, tag="pf", name="pf")
                            for ich in range(2):
                                nc.tensor.matmul(
                                    pf[:],
                                    atn[ich][:, jsl, sbi * 128:(sbi + 1) * 128],
                                    wo_sb[:, ich, osl * 512:(osl + 1) * 512],
                                    start=(ich == 0), stop=(ich == 1),
                                )
                            ob = ost.tile([128, 512], bf16, tag="ob", name="ob")
                            nc.scalar.copy(ob[:], pf[:])
                            nc.sync.dma_start(
                                out_d[sb * 128:(sb + 1) * 128,
                                      osl * 512:(osl + 1) * 512],
                                ob[:],
                            )
    nc.compile()
    return nc


def _host_prep(x, token_positions, WQ, WK, WV, WO):
    """Build the 8 per-core input maps."""
    pos = np.asarray(token_positions).astype(np.float32)
    k = np.arange(DK // 2, dtype=np.float32)
    inv_freq = 1.0 / (THETA ** (2.0 * k / DK))
    ang = pos[:, None] * inv_freq[None, :]          # [S, 32]
    c32 = np.cos(ang).T.astype(np.float32)          # [32, S]
    s32 = np.sin(ang).T.astype(np.float32)
    cosb = np.tile(c32, (4, 1)).astype(BF16)        # [128, S]
    sinb = np.concatenate([-s32, s32, -s32, s32], axis=0).astype(BF16)
    # causal masks for the 4 diagonal key-blocks of a 512-query slice
    kk = np.arange(128)[:, None, None]
    dd = np.arange(4)[None, :, None]
    qq = np.arange(512)[None, None, :]
    msk = np.where(dd * 128 + kk <= qq, 0.0, -1e5).astype(BF16)  # [128, 4, 512]
    eye = np.eye(128, dtype=np.float32).astype(BF16)
    # indicator matrices for denominator broadcast:
    # ind[i, jsl, r] = 1 iff i == (r//64)*4 + jsl  (same for both head pairs)
    ind = np.zeros((40, 4, 128), dtype=np.float32)
    for jsl in range(4):
        for r in range(128):
            ind[(r // 64) * 4 + jsl, jsl, r] = 1.0
            ind[32 + (r // 64) * 4 + jsl, jsl, r] = 1.0
    ind = ind.astype(BF16)

    perm = np.concatenate([np.arange(0, DK, 2), np.arange(1, DK, 2)])  # evens,odds

    in_maps = []
    for core in range(NCORES):
        b, hg = divmod(core, 4)
        ch0 = hg * 256
        qk_rows = np.concatenate([ch0 + hl * 64 + perm for hl in range(HPC)])
        def dev_w(w):  # [D, M] -> [128, 8, M] (contraction chunks)
            return np.ascontiguousarray(
                w.reshape(8, 128, -1).transpose(1, 0, 2)
            ).astype(BF16)

        xt = np.asarray(x[b]).T                       # [D, S]
        xt4 = np.ascontiguousarray(
            xt.reshape(8, 128, 4, 512).transpose(2, 1, 0, 3)
        ).astype(BF16)                                # [4, 128, 8, 512]
        in_maps.append({
            "xt": xt4,
            "wq": dev_w(np.asarray(WQ)[qk_rows, :].T),
            "wk": dev_w(np.asarray(WK)[qk_rows, :].T),
            "wv": dev_w(np.asarray(WV)[ch0:ch0 + 256, :].T),
            "wo": np.ascontiguousarray(
                np.asarray(WO)[:, ch0:ch0 + 256].T.reshape(2, 128, D)
                .transpose(1, 0, 2)
            ).astype(BF16),
            "cosb": cosb,
            "sinb": sinb,
            "msk": msk,
            "eye": eye,
            "ind": ind,
        })
    return in_maps


LAST_EXEC_NS = None
LAST_RES = None


def kernel(x, token_positions, WQ, WK, WV, WO):
    global LAST_EXEC_NS, LAST_RES
    from concourse.bass_utils import run_bass_kernel_spmd

    if "nc" not in _COMPILED:
        _COMPILED["nc"] = _build_nc()
    nc = _COMPILED["nc"]

    in_maps = _host_prep(x, token_positions, WQ, WK, WV, WO)
    res = run_bass_kernel_spmd(nc, in_maps, list(range(NCORES)))
    LAST_EXEC_NS = res.exec_time_ns
    LAST_RES = res

    out = np.zeros((2, S, D), dtype=np.float32)
    for core in range(NCORES):
        out[core // 4] += np.asarray(res.results[core]["out"], dtype=np.float32)
    return out



# revision 5
# speedup vs baseline: 1.0305x; 1.0305x over previous
"""Trainium2 Bass kernel: causal multi-head self-attention with RoPE.

Problem: x[2,2048,1024], 16 heads, d_k=64, causal, RoPE(theta=1e4),
out = (softmax(rope(Q)rope(K)^T/8) V) WO^T.

Sharding (8 cores): data-parallel over batch (2) x head-parallel over
head groups (4 heads per core).  Each core computes Q/K/V projections
for its 4 heads, flash-style causal attention, and a partial output
projection over its 256 channels; the host sums the 4 partials per
batch element.

v2 restructure vs the 209us baseline:
  - scores for the 2 heads of a pair issue as adjacent K=64 matmuls at
    tile_position (0,0)/(64,0) into different PSUM banks -> they run
    CONCURRENTLY on the PE array (row-group packing), halving score cost.
  - causal masking: tri-only eye-matmul preload ([128,128] instead of
    [128,512]) + score matmuls trimmed to cols >= c0 (PV skips them
    anyway); one exp per (pair,kb) covers both heads' [128,1024] psum.
  - j-outer pipeline: attention(pair,j) for both pairs, then per-j
    normalization + output projection; Q/K/V projection + RoPE emitted
    in slices between attention tiles so the Tile scheduler can fill
    TensorE stalls (exp-bound inner loop) with projection matmuls and
    the PE never idles long enough to re-throttle (HAM).
  - output-projection psum->sbuf copies moved from ScalarE to VectorE:
    ScalarE (exp) is the second-binding engine.
Device layouts as baseline: xt pre-chunked [4,128,8,512]; Qt/Kt rows
[32 even,32 odd] per head (host permutes W columns) so RoPE is pure
row-block ops; V [128,16,4,65] with a ones 65th column producing
softmax denominators inside the PV matmul.
"""

import os
import sys

for _p in ("/opt/trn_rl_repo",):
    if _p not in sys.path:
        sys.path.insert(0, _p)

import numpy as np
import ml_dtypes

BF16 = ml_dtypes.bfloat16

D = 1024
S = 2048
H = 16
DK = 64
HPC = 4          # heads per core
NCORES = 8
THETA = 10000.0

_COMPILED = {}


def _build_nc():
    import concourse.bass as bass  # noqa: F401
    import concourse.bacc as bacc
    import concourse.mybir as mybir
    import concourse.tile as tile

    bf16 = mybir.dt.bfloat16
    f32 = mybir.dt.float32
    Exp = mybir.ActivationFunctionType.Exp

    nc = bacc.Bacc(
        "TRN2", target_bir_lowering=False, debug=False, num_devices=NCORES
    )
    xt_d = nc.declare_dram_parameter("xt", [4, 128, 8, 512], bf16, isOutput=False)
    wq_d = nc.declare_dram_parameter("wq", [128, 8, 256], bf16, isOutput=False)
    wk_d = nc.declare_dram_parameter("wk", [128, 8, 256], bf16, isOutput=False)
    wv_d = nc.declare_dram_parameter("wv", [128, 8, 256], bf16, isOutput=False)
    wo_d = nc.declare_dram_parameter("wo", [128, 2, D], bf16, isOutput=False)
    cos_d = nc.declare_dram_parameter("cosb", [128, S], bf16, isOutput=False)
    sin_d = nc.declare_dram_parameter("sinb", [128, S], bf16, isOutput=False)
    msk_d = nc.declare_dram_parameter("msk", [128, 128], bf16, isOutput=False)
    eye_d = nc.declare_dram_parameter("eye", [128, 128], bf16, isOutput=False)
    ind_d = nc.declare_dram_parameter("ind", [40, 4, 128], bf16, isOutput=False)
    out_d = nc.declare_dram_parameter("out", [S, D], bf16, isOutput=True)

    with tile.TileContext(nc) as tc:
        with tc.tile_pool(name="const", bufs=1) as const:
            x_sb = const.tile([128, 8, S], bf16)
            wq_sb = const.tile([128, 8, 256], bf16)
            wk_sb = const.tile([128, 8, 256], bf16)
            wv_sb = const.tile([128, 8, 256], bf16)
            wo_sb = const.tile([128, 2, D], bf16)
            cos_sb = const.tile([128, S], bf16)
            sin_sb = const.tile([128, S], bf16)
            msk_sb = const.tile([128, 128], bf16)
            eye_sb = const.tile([128, 128], bf16)
            ind_sb = const.tile([40, 4, 128], bf16)
            v_sb = const.tile([128, 16, 4, 65], bf16)
            qraw = [const.tile([128, S], bf16, name=f"qraw{i}") for i in range(2)]
            kraw = [const.tile([128, S], bf16, name=f"kraw{i}") for i in range(2)]
            qrot = [const.tile([128, S], bf16, name=f"qrot{i}") for i in range(2)]
            krot = [const.tile([128, S], bf16, name=f"krot{i}") for i in range(2)]
            at = [const.tile([128, S], bf16, name=f"at{i}") for i in range(2)]
            atn = [const.tile([128, 4, 512], bf16, name=f"atn{i}")
                   for i in range(2)]
            den_sb = const.tile([40, 512], bf16)
            denf = const.tile([40, 512], f32)
            rc = const.tile([40, 512], f32)
            rcb = const.tile([40, 512], bf16)

            # x slices stream on the HW-DGE (sync) queue; weights et al on
            # the gpsimd queue, ordered by first use.
            for nsl in range(4):
                nc.sync.dma_start(
                    x_sb[:, :, nsl * 512:(nsl + 1) * 512], xt_d[nsl]
                )
            nc.gpsimd.dma_start(wk_sb[:], wk_d[:])
            nc.gpsimd.dma_start(wq_sb[:], wq_d[:])
            nc.gpsimd.dma_start(cos_sb[:], cos_d[:])
            nc.gpsimd.dma_start(sin_sb[:], sin_d[:])
            nc.gpsimd.dma_start(msk_sb[:], msk_d[:])
            nc.gpsimd.dma_start(eye_sb[:], eye_d[:])
            nc.gpsimd.dma_start(wv_sb[:], wv_d[:])
            nc.gpsimd.dma_start(wo_sb[:], wo_d[:])
            nc.gpsimd.dma_start(ind_sb[:], ind_d[:])
            nc.vector.memset(v_sb[:, :, :, 64:65], 1.0)
            # den rows for not-yet-computed (pair, j) read as 1.0 by the
            # full-tile reciprocal below (engine ops need 32-aligned
            # partition bases, so norm() can't slice 2 rows at a time)
            nc.vector.memset(den_sb[:], 1.0)

            with tc.tile_pool(name="gen", bufs=2, space="PSUM") as gen, \
                 tc.tile_pool(name="scp", bufs=2, space="PSUM") as scp, \
                 tc.tile_pool(name="pop", bufs=2, space="PSUM") as pop, \
                 tc.tile_pool(name="ptp", bufs=4) as ptp, \
                 tc.tile_pool(name="stg", bufs=4) as stg:

                def proj_qk(ot, nsls, w_sb, raw):
                    for nsl in nsls:
                        ps = gen.tile([128, 512], f32, tag="gen", name="pj")
                        for c in range(8):
                            nc.tensor.matmul(
                                ps[:],
                                w_sb[:, c, ot * 128:(ot + 1) * 128],
                                x_sb[:, c, nsl * 512:(nsl + 1) * 512],
                                start=(c == 0), stop=(c == 7),
                            )
                        nc.vector.tensor_copy(
                            raw[ot][:, nsl * 512:(nsl + 1) * 512], ps[:]
                        )

                def rope_half(raw, rot, h):
                    cl = slice(h * 1024, (h + 1) * 1024)
                    sw = stg.tile([128, 1024], bf16, tag="sw", name="sw")
                    t1 = stg.tile([128, 1024], bf16, tag="t1", name="t1")
                    for blk in range(4):
                        src = blk ^ 1
                        nc.sync.dma_start(
                            sw[blk * 32:(blk + 1) * 32, :],
                            raw[src * 32:(src + 1) * 32, cl],
                        )
                    nc.vector.tensor_mul(t1[:], raw[:, cl], cos_sb[:, cl])
                    nc.vector.tensor_mul(sw[:], sw[:], sin_sb[:, cl])
                    nc.vector.tensor_add(rot[:, cl], t1[:], sw[:])

                def proj_v(sbs):
                    for sb in sbs:
                        ps = gen.tile([128, 512], f32, tag="gen", name="pv")
                        for c in range(8):
                            nc.tensor.matmul(
                                ps[:, 0:256],
                                x_sb[:, c, sb * 128:(sb + 1) * 128],
                                wv_sb[:, c, :],
                                start=(c == 0), stop=(c == 7),
                            )
                        nc.vector.tensor_copy(
                            v_sb[:, sb, :, 0:64],
                            ps[:, 0:256].rearrange("p (h d) -> p h d", h=4),
                        )

                def attn(ot, j):
                    qr, kr = qrot[ot], krot[ot]
                    nkb = 4 * (j + 1)
                    hA, hB = 2 * ot, 2 * ot + 1
                    poA = pop.tile([65, 512], f32, tag="po", name="poA")
                    poB = pop.tile([65, 512], f32, tag="po", name="poB")
                    for kb in range(nkb):
                        dg = kb - 4 * j
                        c0 = dg * 128 if dg > 0 else 0
                        sp = scp.tile([128, 1024], f32, tag="sc", name="sp")
                        pt = ptp.tile([128, 1024], bf16, tag="pt", name="pt")
                        if dg >= 0:
                            # tri-only causal mask preload: start=True clears
                            # the bank's has_written bits, so the score matmul
                            # (start=False) accumulates on the tri region and
                            # overwrites the rest
                            nc.tensor.matmul(
                                sp[:, c0:c0 + 128], eye_sb[:], msk_sb[:],
                                start=True, stop=False,
                            )
                            nc.tensor.matmul(
                                sp[:, 512 + c0:512 + c0 + 128],
                                eye_sb[:], msk_sb[:],
                                start=True, stop=False,
                            )
                        # 2-head packed score matmuls (K=64 row groups 0/64)
                        nc.tensor.matmul(
                            sp[:, c0:512],
                            kr[0:64, kb * 128:(kb + 1) * 128],
                            qr[0:64, j * 512 + c0:(j + 1) * 512],
                            start=(dg < 0), stop=True,
                        )
                        nc.tensor.matmul(
                            sp[:, 512 + c0:1024],
                            kr[64:128, kb * 128:(kb + 1) * 128],
                            qr[64:128, j * 512 + c0:(j + 1) * 512],
                            start=(dg < 0), stop=True,
                        )
                        nc.scalar.activation(pt[:], sp[:], Exp, scale=0.125)
                        nc.tensor.matmul(
                            poA[:, c0:512],
                            v_sb[:, kb, hA, 0:65],
                            pt[:, c0:512],
                            start=(kb == 0), stop=(kb == nkb - 1),
                        )
                        nc.tensor.matmul(
                            poB[:, c0:512],
                            v_sb[:, kb, hB, 0:65],
                            pt[:, 512 + c0:1024],
                            start=(kb == 0), stop=(kb == nkb - 1),
                        )
                    for hl, po in ((0, poA), (1, poB)):
                        tm = stg.tile([65, 512], bf16, tag="tm", name="tm")
                        nc.vector.tensor_copy(tm[:], po[:])
                        r0 = hl * 64
                        nc.sync.dma_start(
                            at[ot][r0:r0 + 64, j * 512:(j + 1) * 512],
                            tm[0:64, :],
                        )
                        dr = ot * 32 + j * 2 + hl
                        nc.sync.dma_start(den_sb[dr:dr + 1, :], tm[64:65, :])

                def norm(j):
                    nc.vector.tensor_copy(denf[:], den_sb[:])
                    nc.vector.reciprocal_approx_fast(rc[:], denf[:])
                    nc.vector.tensor_copy(rcb[:], rc[:])
                    for ot in range(2):
                        rbp = gen.tile([128, 512], f32, tag="gen", name="rb")
                        nc.tensor.matmul(
                            rbp[:], ind_sb[ot * 32:ot * 32 + 8, j, :],
                            rcb[ot * 32:ot * 32 + 8, :], start=True, stop=True,
                        )
                        nc.vector.tensor_mul(
                            atn[ot][:, j, :],
                            at[ot][:, j * 512:(j + 1) * 512],
                            rbp[:],
                        )

                def outproj(j):
                    for sbi in range(4):
                        sb = j * 4 + sbi
                        for osl in range(2):
                            pf = gen.tile([128, 512], f32, tag="gen", name="pf")
                            for ich in range(2):
                                nc.tensor.matmul(
                                    pf[:],
                                    atn[ich][:, j, sbi * 128:(sbi + 1) * 128],
                                    wo_sb[:, ich, osl * 512:(osl + 1) * 512],
                                    start=(ich == 0), stop=(ich == 1),
                                )
                            ob = stg.tile([128, 512], bf16, tag="ob", name="ob")
                            nc.vector.tensor_copy(ob[:], pf[:])
                            nc.gpsimd.dma_start(
                                out_d[sb * 128:(sb + 1) * 128,
                                      osl * 512:(osl + 1) * 512],
                                ob[:],
                            )

                # ---- emission order == scheduler priority ----
                proj_qk(0, [0, 1], wk_sb, kraw)
                proj_qk(0, [0, 1], wq_sb, qraw)
                rope_half(kraw[0], krot[0], 0)
                rope_half(qraw[0], qrot[0], 0)
                proj_v(range(0, 8))
                proj_qk(1, [0, 1], wk_sb, kraw)
                proj_qk(1, [0, 1], wq_sb, qraw)
                rope_half(kraw[1], krot[1], 0)
                rope_half(qraw[1], qrot[1], 0)
                attn(0, 0)
                attn(1, 0)
                proj_qk(0, [2, 3], wk_sb, kraw)
                rope_half(kraw[0], krot[0], 1)
                proj_qk(0, [2, 3], wq_sb, qraw)
                rope_half(qraw[0], qrot[0], 1)
                norm(0)
                attn(0, 1)
                attn(1, 1)
                proj_v(range(8, 16))
                proj_qk(1, [2, 3], wk_sb, kraw)
                rope_half(kraw[1], krot[1], 1)
                proj_qk(1, [2, 3], wq_sb, qraw)
                rope_half(qraw[1], qrot[1], 1)
                outproj(0)
                norm(1)
                attn(0, 2)
                attn(1, 2)
                norm(2)
                outproj(1)
                attn(0, 3)
                attn(1, 3)
                norm(3)
                outproj(2)
                outproj(3)
    nc.compile()
    return nc


def _host_prep(x, token_positions, WQ, WK, WV, WO):
    """Build the 8 per-core input maps."""
    pos = np.asarray(token_positions).astype(np.float32)
    k = np.arange(DK // 2, dtype=np.float32)
    inv_freq = 1.0 / (THETA ** (2.0 * k / DK))
    ang = pos[:, None] * inv_freq[None, :]          # [S, 32]
    c32 = np.cos(ang).T.astype(np.float32)          # [32, S]
    s32 = np.sin(ang).T.astype(np.float32)
    cosb = np.tile(c32, (4, 1)).astype(BF16)        # [128, S]
    sinb = np.concatenate([-s32, s32, -s32, s32], axis=0).astype(BF16)
    # tri causal mask for the 128-wide diagonal sub-block: key k > query q
    kk = np.arange(128)[:, None]
    qq = np.arange(128)[None, :]
    msk = np.where(kk <= qq, 0.0, -1e5).astype(BF16)  # [128, 128]
    eye = np.eye(128, dtype=np.float32).astype(BF16)
    # indicator matrices for denominator broadcast:
    # ind[ot*32 + i, j, r] = 1 iff i == j*2 + (r//64)
    ind = np.zeros((40, 4, 128), dtype=np.float32)
    for j in range(4):
        for r in range(128):
            ind[j * 2 + (r // 64), j, r] = 1.0
            ind[32 + j * 2 + (r // 64), j, r] = 1.0
    ind = ind.astype(BF16)

    perm = np.concatenate([np.arange(0, DK, 2), np.arange(1, DK, 2)])  # evens,odds

    in_maps = []
    for core in range(NCORES):
        b, hg = divmod(core, 4)
        ch0 = hg * 256
        qk_rows = np.concatenate([ch0 + hl * 64 + perm for hl in range(HPC)])
        def dev_w(w):  # [D, M] -> [128, 8, M] (contraction chunks)
            return np.ascontiguousarray(
                w.reshape(8, 128, -1).transpose(1, 0, 2)
            ).astype(BF16)

        xt = np.asarray(x[b]).T                       # [D, S]
        xt4 = np.ascontiguousarray(
            xt.reshape(8, 128, 4, 512).transpose(2, 1, 0, 3)
        ).astype(BF16)                                # [4, 128, 8, 512]
        in_maps.append({
            "xt": xt4,
            "wq": dev_w(np.asarray(WQ)[qk_rows, :].T),
            "wk": dev_w(np.asarray(WK)[qk_rows, :].T),
            "wv": dev_w(np.asarray(WV)[ch0:ch0 + 256, :].T),
            "wo": np.ascontiguousarray(
                np.asarray(WO)[:, ch0:ch0 + 256].T.reshape(2, 128, D)
                .transpose(1, 0, 2)
            ).astype(BF16),
            "cosb": cosb,
            "sinb": sinb,
            "msk": msk,
            "eye": eye,
            "ind": ind,
        })
    return in_maps


LAST_EXEC_NS = None
LAST_RES = None


def kernel(x, token_positions, WQ, WK, WV, WO):
    global LAST_EXEC_NS, LAST_RES
    from concourse.bass_utils import run_bass_kernel_spmd

    if "nc" not in _COMPILED:
        _COMPILED["nc"] = _build_nc()
    nc = _COMPILED["nc"]

    in_maps = _host_prep(x, token_positions, WQ, WK, WV, WO)
    res = run_bass_kernel_spmd(nc, in_maps, list(range(NCORES)))
    LAST_EXEC_NS = res.exec_time_ns
    LAST_RES = res

    out = np.zeros((2, S, D), dtype=np.float32)
    for core in range(NCORES):
        out[core // 4] += np.asarray(res.results[core]["out"], dtype=np.float32)
    return out


# revision 6
# speedup vs baseline: 1.1974x; 1.1619x over previous
"""Trainium2 Bass kernel: causal multi-head self-attention with RoPE.

Problem: x[2,2048,1024], 16 heads, d_k=64, causal, RoPE(theta=1e4),
out = (softmax(rope(Q)rope(K)^T/8) V) WO^T.

Sharding (8 cores): data-parallel over batch (2) x head-parallel over
head groups (4 heads per core).  Each core computes Q/K/V projections
for its 4 heads, flash-style causal attention, and a partial output
projection over its 256 channels; the host sums the 4 partials per
batch element.

v3 structure:
  - scores for the 2 heads of a pair issue as adjacent K=64 matmuls at
    row groups 0/64 into different PSUM banks -> concurrent on the PE.
  - causal masking post-exp on GpSimd (0/1 tri multiply on the 128-wide
    diagonal sub-block) - no eye/mask matmuls on TensorE at all; score
    matmuls and exp are trimmed to cols >= c0 (PV skips them anyway).
  - Q/K projection and output projection reuse the stationary operand:
    explicit ldweights + two ldweights=False matmuls into alternating
    PSUM banks (walrus is compiled with ldw-opt off, so every
    self-loading matmul pays an exposed LDWEIGHTS + drain ~2x cost).
  - j-outer pipeline: attention(pair,j) for both pairs, then per-j
    normalization + output projection; projection/RoPE emitted in
    slices between attention tiles so the Tile scheduler can fill
    TensorE stalls and the PE never re-throttles (HAM).
  - PSUM->SBUF evacuations split across VectorE and ScalarE by phase
    (ScalarE takes the ones in its exp-idle windows).
Device layouts as baseline: xt pre-chunked [4,128,8,512]; Qt/Kt rows
[32 even,32 odd] per head (host permutes W columns) so RoPE is pure
row-block ops; V [128,16,4,65] with a ones 65th column producing
softmax denominators inside the PV matmul.
"""

import os
import sys

for _p in ("/opt/trn_rl_repo",):
    if _p not in sys.path:
        sys.path.insert(0, _p)

import numpy as np
import ml_dtypes

BF16 = ml_dtypes.bfloat16

D = 1024
S = 2048
H = 16
DK = 64
HPC = 4          # heads per core
NCORES = 8
THETA = 10000.0

_COMPILED = {}


def _build_nc():
    import concourse.bass as bass  # noqa: F401
    import concourse.bacc as bacc
    import concourse.mybir as mybir
    import concourse.tile as tile

    bf16 = mybir.dt.bfloat16
    f32 = mybir.dt.float32
    Exp = mybir.ActivationFunctionType.Exp

    nc = bacc.Bacc(
        "TRN2", target_bir_lowering=False, debug=False, num_devices=NCORES
    )
    xt_d = nc.declare_dram_parameter("xt", [4, 128, 8, 512], bf16, isOutput=False)
    wq_d = nc.declare_dram_parameter("wq", [128, 8, 256], bf16, isOutput=False)
    wk_d = nc.declare_dram_parameter("wk", [128, 8, 256], bf16, isOutput=False)
    wv_d = nc.declare_dram_parameter("wv", [128, 8, 256], bf16, isOutput=False)
    wo_d = nc.declare_dram_parameter("wo", [128, 2, D], bf16, isOutput=False)
    cos_d = nc.declare_dram_parameter("cosb", [128, S], bf16, isOutput=False)
    sin_d = nc.declare_dram_parameter("sinb", [128, S], bf16, isOutput=False)
    msk_d = nc.declare_dram_parameter("msk", [128, 128], bf16, isOutput=False)
    ind_d = nc.declare_dram_parameter("ind", [40, 4, 128], bf16, isOutput=False)
    out_d = nc.declare_dram_parameter("out", [S, D], bf16, isOutput=True)

    def noldw(*mms):
        for mm in mms:
            mm.ins.ldweights = False

    with tile.TileContext(nc) as tc:
        with tc.tile_pool(name="const", bufs=1) as const:
            x_sb = const.tile([128, 8, S], bf16)
            wq_sb = const.tile([128, 8, 256], bf16)
            wk_sb = const.tile([128, 8, 256], bf16)
            wv_sb = const.tile([128, 8, 256], bf16)
            wo_sb = const.tile([128, 2, D], bf16)
            cos_sb = const.tile([128, S], bf16)
            sin_sb = const.tile([128, S], bf16)
            msk_sb = const.tile([128, 128], bf16)
            ind_sb = const.tile([40, 4, 128], bf16)
            v_sb = const.tile([128, 16, 4, 65], bf16)
            qraw = [const.tile([128, S], bf16, name=f"qraw{i}") for i in range(2)]
            kraw = [const.tile([128, S], bf16, name=f"kraw{i}") for i in range(2)]
            qrot = [const.tile([128, S], bf16, name=f"qrot{i}") for i in range(2)]
            krot = [const.tile([128, S], bf16, name=f"krot{i}") for i in range(2)]
            at = [const.tile([128, S], bf16, name=f"at{i}") for i in range(2)]
            atn = [const.tile([128, 4, 512], bf16, name=f"atn{i}")
                   for i in range(2)]
            den_sb = const.tile([40, 512], bf16)
            denf = const.tile([40, 512], f32)
            rc = const.tile([40, 512], f32)
            rcb = const.tile([40, 512], bf16)

            # sync (HW DGE, fast): weights/x in first-use order, then the
            # per-phase rope-swap and at-staging transfers.
            nc.sync.dma_start(wk_sb[:], wk_d[:])
            nc.sync.dma_start(wq_sb[:], wq_d[:])
            nc.sync.dma_start(x_sb[:, :, 0:512], xt_d[0])
            nc.sync.dma_start(x_sb[:, :, 512:1024], xt_d[1])
            nc.sync.dma_start(cos_sb[:], cos_d[:])
            nc.sync.dma_start(sin_sb[:], sin_d[:])
            nc.sync.dma_start(msk_sb[:], msk_d[:])
            nc.sync.dma_start(x_sb[:, :, 1024:1536], xt_d[2])
            nc.sync.dma_start(x_sb[:, :, 1536:2048], xt_d[3])
            # gpsimd queue: the rest, plus den staging and out stores later
            nc.gpsimd.dma_start(wv_sb[:], wv_d[:])
            nc.gpsimd.dma_start(wo_sb[:], wo_d[:])
            nc.gpsimd.dma_start(ind_sb[:], ind_d[:])
            nc.vector.memset(v_sb[:, :, :, 64:65], 1.0)
            # den rows for not-yet-computed (pair, j) read as 1.0 by the
            # full-tile reciprocal in norm()
            nc.vector.memset(den_sb[:], 1.0)

            with tc.tile_pool(name="gen", bufs=2, space="PSUM") as gen, \
                 tc.tile_pool(name="scp", bufs=2, space="PSUM") as scp, \
                 tc.tile_pool(name="pop", bufs=2, space="PSUM") as pop, \
                 tc.tile_pool(name="ptp", bufs=4) as ptp, \
                 tc.tile_pool(name="stg", bufs=4) as stg:

                def proj_qk(ot, nsls, w_sb, raw, cast_eng):
                    # paired-nsl with stationary-weight reuse: one explicit
                    # ldweights serves two matmuls into alternating banks
                    na, nb = nsls
                    pa = gen.tile([128, 512], f32, tag="gen", name="pja")
                    pb = gen.tile([128, 512], f32, tag="gen", name="pjb")
                    for c in range(8):
                        w_ap = w_sb[:, c, ot * 128:(ot + 1) * 128]
                        nc.tensor.ldweights(w_ap)
                        m1 = nc.tensor.matmul(
                            pa[:], w_ap, x_sb[:, c, na * 512:(na + 1) * 512],
                            start=(c == 0), stop=(c == 7),
                        )
                        m2 = nc.tensor.matmul(
                            pb[:], w_ap, x_sb[:, c, nb * 512:(nb + 1) * 512],
                            start=(c == 0), stop=(c == 7),
                        )
                        noldw(m1, m2)
                    for ps, nsl in ((pa, na), (pb, nb)):
                        if cast_eng == "scalar":
                            nc.scalar.copy(
                                raw[ot][:, nsl * 512:(nsl + 1) * 512], ps[:]
                            )
                        else:
                            nc.vector.tensor_copy(
                                raw[ot][:, nsl * 512:(nsl + 1) * 512], ps[:]
                            )

                def rope_half(raw, rot, h):
                    cl = slice(h * 1024, (h + 1) * 1024)
                    sw = stg.tile([128, 1024], bf16, tag="sw", name="sw")
                    t1 = stg.tile([128, 1024], bf16, tag="t1", name="t1")
                    for blk in range(4):
                        src = blk ^ 1
                        nc.sync.dma_start(
                            sw[blk * 32:(blk + 1) * 32, :],
                            raw[src * 32:(src + 1) * 32, cl],
                        )
                    nc.vector.tensor_mul(t1[:], raw[:, cl], cos_sb[:, cl])
                    nc.vector.tensor_mul(sw[:], sw[:], sin_sb[:, cl])
                    nc.vector.tensor_add(rot[:, cl], t1[:], sw[:])

                def proj_v(sbs, cast_eng):
                    for sb in sbs:
                        ps = gen.tile([128, 512], f32, tag="gen", name="pv")
                        for c in range(8):
                            nc.tensor.matmul(
                                ps[:, 0:256],
                                x_sb[:, c, sb * 128:(sb + 1) * 128],
                                wv_sb[:, c, :],
                                start=(c == 0), stop=(c == 7),
                            )
                        src = ps[:, 0:256].rearrange("p (h d) -> p h d", h=4)
                        if cast_eng == "scalar":
                            nc.scalar.copy(v_sb[:, sb, :, 0:64], src)
                        else:
                            nc.vector.tensor_copy(v_sb[:, sb, :, 0:64], src)

                def attn(ot, j):
                    qr, kr = qrot[ot], krot[ot]
                    nkb = 4 * (j + 1)
                    hA, hB = 2 * ot, 2 * ot + 1
                    poA = pop.tile([65, 512], f32, tag="po", name="poA")
                    poB = pop.tile([65, 512], f32, tag="po", name="poB")
                    for kb in range(nkb):
                        dg = kb - 4 * j
                        c0 = dg * 128 if dg > 0 else 0
                        sp = scp.tile([128, 1024], f32, tag="sc", name="sp")
                        pt = ptp.tile([128, 1024], bf16, tag="pt", name="pt")
                        # 2-head packed score matmuls (K=64 row groups 0/64)
                        nc.tensor.matmul(
                            sp[:, c0:512],
                            kr[0:64, kb * 128:(kb + 1) * 128],
                            qr[0:64, j * 512 + c0:(j + 1) * 512],
                            start=True, stop=True,
                        )
                        nc.tensor.matmul(
                            sp[:, 512 + c0:1024],
                            kr[64:128, kb * 128:(kb + 1) * 128],
                            qr[64:128, j * 512 + c0:(j + 1) * 512],
                            start=True, stop=True,
                        )
                        nc.scalar.activation(
                            pt[:, c0:1024], sp[:, c0:1024], Exp, scale=0.125
                        )
                        if dg >= 0:
                            # causal mask: zero the exp'd upper-tri of the
                            # 128-wide diagonal sub-block on GpSimd
                            nc.gpsimd.tensor_mul(
                                pt[:, c0:c0 + 128], pt[:, c0:c0 + 128],
                                msk_sb[:],
                            )
                            nc.gpsimd.tensor_mul(
                                pt[:, 512 + c0:512 + c0 + 128],
                                pt[:, 512 + c0:512 + c0 + 128],
                                msk_sb[:],
                            )
                        nc.tensor.matmul(
                            poA[:, c0:512],
                            v_sb[:, kb, hA, 0:65],
                            pt[:, c0:512],
                            start=(kb == 0), stop=(kb == nkb - 1),
                        )
                        nc.tensor.matmul(
                            poB[:, c0:512],
                            v_sb[:, kb, hB, 0:65],
                            pt[:, 512 + c0:1024],
                            start=(kb == 0), stop=(kb == nkb - 1),
                        )
                    for hl, po in ((0, poA), (1, poB)):
                        tm = stg.tile([65, 512], bf16, tag="tm", name="tm")
                        nc.vector.tensor_copy(tm[:], po[:])
                        r0 = hl * 64
                        nc.sync.dma_start(
                            at[ot][r0:r0 + 64, j * 512:(j + 1) * 512],
                            tm[0:64, :],
                        )
                        dr = ot * 32 + j * 2 + hl
                        nc.gpsimd.dma_start(den_sb[dr:dr + 1, :], tm[64:65, :])

                def norm(j):
                    nc.vector.tensor_copy(denf[:], den_sb[:])
                    nc.vector.reciprocal_approx_fast(rc[:], denf[:])
                    nc.vector.tensor_copy(rcb[:], rc[:])
                    for ot in range(2):
                        rbp = gen.tile([128, 512], f32, tag="gen", name="rb")
                        nc.tensor.matmul(
                            rbp[:], ind_sb[ot * 32:ot * 32 + 8, j, :],
                            rcb[ot * 32:ot * 32 + 8, :], start=True, stop=True,
                        )
                        nc.vector.tensor_mul(
                            atn[ot][:, j, :],
                            at[ot][:, j * 512:(j + 1) * 512],
                            rbp[:],
                        )

                def outproj(j, cast_eng="vector"):
                    for sbi in range(4):
                        sb = j * 4 + sbi
                        pf0 = gen.tile([128, 512], f32, tag="gen", name="pf0")
                        pf1 = gen.tile([128, 512], f32, tag="gen", name="pf1")
                        for ich in range(2):
                            a_ap = atn[ich][:, j, sbi * 128:(sbi + 1) * 128]
                            nc.tensor.ldweights(a_ap)
                            m1 = nc.tensor.matmul(
                                pf0[:], a_ap, wo_sb[:, ich, 0:512],
                                start=(ich == 0), stop=(ich == 1),
                            )
                            m2 = nc.tensor.matmul(
                                pf1[:], a_ap, wo_sb[:, ich, 512:1024],
                                start=(ich == 0), stop=(ich == 1),
                            )
                            noldw(m1, m2)
                        for osl, pf in ((0, pf0), (1, pf1)):
                            ob = stg.tile([128, 512], bf16, tag="ob", name="ob")
                            if cast_eng == "scalar":
                                nc.scalar.copy(ob[:], pf[:])
                            else:
                                nc.vector.tensor_copy(ob[:], pf[:])
                            dma_q = nc.sync if osl == 0 else nc.gpsimd
                            dma_q.dma_start(
                                out_d[sb * 128:(sb + 1) * 128,
                                      osl * 512:(osl + 1) * 512],
                                ob[:],
                            )

                # ---- emission order == scheduler priority ----
                proj_qk(0, (0, 1), wk_sb, kraw, "scalar")
                rope_half(kraw[0], krot[0], 0)
                proj_qk(0, (0, 1), wq_sb, qraw, "scalar")
                rope_half(qraw[0], qrot[0], 0)
                proj_v(range(0, 8), "scalar")
                proj_qk(1, (0, 1), wk_sb, kraw, "scalar")
                rope_half(kraw[1], krot[1], 0)
                proj_qk(1, (0, 1), wq_sb, qraw, "scalar")
                rope_half(qraw[1], qrot[1], 0)
                attn(0, 0)
                attn(1, 0)
                proj_qk(0, (2, 3), wk_sb, kraw, "vector")
                rope_half(kraw[0], krot[0], 1)
                proj_qk(0, (2, 3), wq_sb, qraw, "vector")
                rope_half(qraw[0], qrot[0], 1)
                norm(0)
                attn(0, 1)
                attn(1, 1)
                proj_v(range(8, 16), "vector")
                proj_qk(1, (2, 3), wk_sb, kraw, "vector")
                rope_half(kraw[1], krot[1], 1)
                proj_qk(1, (2, 3), wq_sb, qraw, "vector")
                rope_half(qraw[1], qrot[1], 1)
                outproj(0)
                norm(1)
                attn(0, 2)
                attn(1, 2)
                norm(2)
                outproj(1)
                attn(0, 3)
                attn(1, 3)
                norm(3)
                outproj(2)
                outproj(3, "scalar")
    nc.compile()
    return nc


def _host_prep(x, token_positions, WQ, WK, WV, WO):
    """Build the 8 per-core input maps."""
    pos = np.asarray(token_positions).astype(np.float32)
    k = np.arange(DK // 2, dtype=np.float32)
    inv_freq = 1.0 / (THETA ** (2.0 * k / DK))
    ang = pos[:, None] * inv_freq[None, :]          # [S, 32]
    c32 = np.cos(ang).T.astype(np.float32)          # [32, S]
    s32 = np.sin(ang).T.astype(np.float32)
    cosb = np.tile(c32, (4, 1)).astype(BF16)        # [128, S]
    sinb = np.concatenate([-s32, s32, -s32, s32], axis=0).astype(BF16)
    # 0/1 keep-mask for the 128-wide diagonal sub-block: keep key k <= query q
    kk = np.arange(128)[:, None]
    qq = np.arange(128)[None, :]
    msk = np.where(kk <= qq, 1.0, 0.0).astype(BF16)  # [128, 128]
    # indicator matrices for denominator broadcast:
    # ind[ot*32 + i, j, r] = 1 iff i == j*2 + (r//64)
    ind = np.zeros((40, 4, 128), dtype=np.float32)
    for j in range(4):
        for r in range(128):
            ind[j * 2 + (r // 64), j, r] = 1.0
            ind[32 + j * 2 + (r // 64), j, r] = 1.0
    ind = ind.astype(BF16)

    perm = np.concatenate([np.arange(0, DK, 2), np.arange(1, DK, 2)])  # evens,odds

    in_maps = []
    for core in range(NCORES):
        b, hg = divmod(core, 4)
        ch0 = hg * 256
        qk_rows = np.concatenate([ch0 + hl * 64 + perm for hl in range(HPC)])
        def dev_w(w):  # [D, M] -> [128, 8, M] (contraction chunks)
            return np.ascontiguousarray(
                w.reshape(8, 128, -1).transpose(1, 0, 2)
            ).astype(BF16)

        xt = np.asarray(x[b]).T                       # [D, S]
        xt4 = np.ascontiguousarray(
            xt.reshape(8, 128, 4, 512).transpose(2, 1, 0, 3)
        ).astype(BF16)                                # [4, 128, 8, 512]
        in_maps.append({
            "xt": xt4,
            "wq": dev_w(np.asarray(WQ)[qk_rows, :].T),
            "wk": dev_w(np.asarray(WK)[qk_rows, :].T),
            "wv": dev_w(np.asarray(WV)[ch0:ch0 + 256, :].T),
            "wo": np.ascontiguousarray(
                np.asarray(WO)[:, ch0:ch0 + 256].T.reshape(2, 128, D)
                .transpose(1, 0, 2)
            ).astype(BF16),
            "cosb": cosb,
            "sinb": sinb,
            "msk": msk,
            "ind": ind,
        })
    return in_maps


LAST_EXEC_NS = None
LAST_RES = None


def kernel(x, token_positions, WQ, WK, WV, WO):
    global LAST_EXEC_NS, LAST_RES
    from concourse.bass_utils import run_bass_kernel_spmd

    if "nc" not in _COMPILED:
        _COMPILED["nc"] = _build_nc()
    nc = _COMPILED["nc"]

    in_maps = _host_prep(x, token_positions, WQ, WK, WV, WO)
    res = run_bass_kernel_spmd(nc, in_maps, list(range(NCORES)))
    LAST_EXEC_NS = res.exec_time_ns
    LAST_RES = res

    out = np.zeros((2, S, D), dtype=np.float32)
    for core in range(NCORES):
        out[core // 4] += np.asarray(res.results[core]["out"], dtype=np.float32)
    return out
